# revision 39
# baseline (speedup 1.0000x reference)
"""BaseLayer MoE gate (balanced assignment) for Trainium2, 8 NeuronCores.

Strategy:
  - The roofline-dominant work is the token->expert affinity matmul
    X[16384, 2048] @ C.T[2048, 16] (reads 134 MB; the kernel is
    HBM-bandwidth bound).  Tokens are sharded 8 ways; each core computes
    aff.T[16, 2048] for its 2048-token shard.
  - Per core: X-shard is fed pre-transposed ([d_model, tok], so the
    d_model contraction lands on SBUF partitions) and streamed as
    sixteen 1MB DMA loads on ONE HWDGE queue (strict FIFO: chunk k
    completes every ~2.5us, so the PE wave for chunk k runs while
    chunk k+1 streams; after the last byte only the last wave + a
    short evacuation remain on the critical path).  The fp32 matmul
    uses PE *column tiling* (tile_position=(0, 32b)) to run the four
    512-token blocks concurrently in the four 32-column PE quadrants
    (fp32 moving costs 4 cycles/row, so without packing the PE would
    be the bottleneck).  Contraction accumulates over 16 k-chunks into
    one PSUM bank.
  - Evacuation: ONE [128, 512] PSUM->SBUF vector copy (DVE time is
    free-dim bound, so copying the whole bank costs the same as one
    quadrant) followed by ONE 256KB store on the drained input queue;
    the host discards the padding rows between quadrants for free.
    A tiny warm-up DMA absorbs the ~1.1us cold-start ramp of the first
    packet on every SDMA engine, and the DCE-keepalive sink store is
    512B so it does not displace input packets on engine 0 (whose
    mid-stream runtime table-refill hiccup already makes it the
    straggler that gates the final chunk).
  - fp32 precision end-to-end is required: the auction's final
    assignment is stable under affinity perturbations up to ~1e-6 but
    flips thousands of indices by 1e-5, which rules out bf16/fp32r
    tricks (verified empirically).
  - Warm-keeping dummy matmuls (one per mid-stream chunk, scratch PSUM
    bank): the PE clock governor (HAM) throttles during long DMA
    waits; the dummies keep the PE boosted so the final (critical-path)
    wave runs at the fast clock.
  - The auction-based balanced assignment operates on the tiny
    [16, 16384] affinity matrix and is an inherently sequential,
    data-dependent while loop (converges in ~11 iterations here); it
    runs on host as an exact bit-level replica of the reference
    semantics (verified to reproduce jax.lax.top_k tie-breaking and the
    full reference trajectory).
"""

import numpy as np

D = 2048
E = 16
N_CORES = 8
TOK_PER_CORE = 2048
N_TOK = N_CORES * TOK_PER_CORE
TOK_BLK = 512
N_BLK = TOK_PER_CORE // TOK_BLK  # 4
K_CHUNKS = D // 128  # 16

_cache = {}


def _build_nc():
    import concourse.tile as tile
    from concourse import bacc, mybir

    f32 = mybir.dt.float32

    nc = bacc.Bacc(
        "TRN2", target_bir_lowering=False, debug=False, num_devices=N_CORES
    )
    xt = nc.declare_dram_parameter("xt", [D, TOK_PER_CORE], f32, isOutput=False)
    # ctp: centroids pre-arranged on host as [128, K_CHUNKS, E]
    ctp = nc.declare_dram_parameter("ctp", [128, K_CHUNKS, E], f32, isOutput=False)
    # raw PSUM-layout output: row 32*b + e holds aff[e, 512b:512b+512]
    # (rows 16..31 of each 32-row quadrant are don't-care padding)
    afft_pad = nc.declare_dram_parameter(
        "afft_pad", [128, TOK_BLK], f32, isOutput=True
    )
    # internal sink that keeps the warm-up dummy matmuls live past DCE;
    # 512B so its store is one descriptor on one SDMA engine (a 32KB sink
    # displaced ~1.5us of engine 0's input stream, measurably gating the
    # final chunk's completion)
    sink = nc.dram_tensor("sink", [E, 8], f32)

    with tile.TileContext(nc) as tc:
        with tc.tile_pool(name="cpool", bufs=1) as cpool, \
             tc.tile_pool(name="xpool", bufs=K_CHUNKS - 2) as xpool, \
             tc.tile_pool(name="lpool", bufs=1) as lpool, \
             tc.tile_pool(name="opool", bufs=1) as opool, \
             tc.tile_pool(name="spool", bufs=1) as spool, \
             tc.tile_pool(name="psum", bufs=1, space="PSUM") as psum_pool, \
             tc.tile_pool(name="psum2", bufs=2, space="PSUM") as psum2_pool:
            ct_sb = cpool.tile([128, K_CHUNKS, E], f32)
            nc.scalar.dma_start(out=ct_sb[:], in_=ctp[:])
            # One PSUM bank [128, TOK_BLK]; col tile b owns partitions
            # 32b..32b+E (M=16 rows of its 32-partition quadrant).
            ps = psum_pool.tile([128, TOK_BLK], f32)

            # Tiny warm-up transfer: the first packet of a cold HWDGE queue
            # runs ~1.1us instead of ~0.3us on every engine; absorb that
            # ramp on an 8KB throwaway so the real stream starts at line
            # rate.
            warm = spool.tile([128, 16], f32, tag="warm", name="warm")
            nc.sync.dma_start(out=warm[:], in_=xt[0:128, 0:16])

            dummies = []
            for k in range(K_CHUNKS - 2):
                xk = xpool.tile([128, TOK_PER_CORE], f32, tag="xk", name=f"xk_{k}")
                nc.sync.dma_start(out=xk[:], in_=xt[k * 128:(k + 1) * 128, :])
                for b in range(N_BLK):
                    nc.tensor.matmul(
                        ps[32 * b:32 * b + E, :],
                        ct_sb[:, k, :],
                        xk[:, b * TOK_BLK:(b + 1) * TOK_BLK],
                        start=(k == 0), stop=False,
                        tile_position=(0, 32 * b),
                    )
                if 1 <= k <= K_CHUNKS - 3:
                    # Light warm-keeping dummy matmul (see module
                    # docstring): 128 moving rows is enough to keep HAM
                    # boosted but leaves the PE ~1.8us of slack per chunk,
                    # so the PE never builds a backlog.
                    ps2 = psum2_pool.tile(
                        [128, TOK_BLK], f32, tag="ps2", name=f"ps2_{k}"
                    )
                    nc.tensor.matmul(
                        ps2[0:E, 0:128],
                        ct_sb[:, k, :],
                        xk[:, 0:128],
                        start=True, stop=True,
                        tile_position=(0, 0),
                    )
                    dummies.append(ps2)
                if k == 1:
                    # anchor the DCE-keepalive to the FIRST dummy bank so
                    # this chain retires mid-stream instead of extending
                    # the kernel tail.
                    sb = spool.tile([E, 8], f32, tag="sb", name="sb")
                    nc.vector.tensor_copy(sb[:], dummies[0][0:E, 0:8])
                    nc.scalar.dma_start(out=sink[:], in_=sb[:])

            # The last TWO chunks ride ONE 2MB rearranged transfer with a
            # single completion semaphore, so the tail pays the
            # receipt+sem-wake latency once instead of twice (both final
            # waves are gated by straggling engine 0 either way), and no
            # dummy sits between the two final waves.
            xk_l = lpool.tile(
                [128, 2, TOK_PER_CORE], f32, tag="xk_l", name="xk_last"
            )
            src = xt[
                (K_CHUNKS - 2) * 128:K_CHUNKS * 128, :
            ].rearrange("(kk p) t -> p kk t", kk=2)
            nc.sync.dma_start(out=xk_l[:], in_=src)
            for kk in range(2):
                k = K_CHUNKS - 2 + kk
                for b in range(N_BLK):
                    nc.tensor.matmul(
                        ps[32 * b:32 * b + E, :],
                        ct_sb[:, k, :],
                        xk_l[:, kk, b * TOK_BLK:(b + 1) * TOK_BLK],
                        start=False, stop=(k == K_CHUNKS - 1),
                        tile_position=(0, 32 * b),
                    )

            # Evacuate the whole PSUM bank in ONE [128, 512] vector copy
            # (DVE time is free-dim bound, so this costs the same as one
            # 16-partition copy), then a single 256KB store on the
            # (now-drained) sync queue; the host discards the padding.
            ob = opool.tile([128, TOK_BLK], f32, tag="ob", name="ob")
            nc.vector.tensor_copy(ob[:], ps[:])
            nc.sync.dma_start(out=afft_pad[:], in_=ob[:])
    nc.compile()
    return nc


def _get_nc():
    if "nc" not in _cache:
        _cache["nc"] = _build_nc()
    return _cache["nc"]


def _make_in_maps(x_flat, centroids):
    # [E, D] -> C.T [D, E] -> [K_CHUNKS, 128, E] -> [128, K_CHUNKS, E]
    ctp = np.ascontiguousarray(
        centroids.T.astype(np.float32, copy=False)
        .reshape(K_CHUNKS, 128, E)
        .transpose(1, 0, 2)
    )
    in_maps = []
    for i in range(N_CORES):
        shard = x_flat[i * TOK_PER_CORE:(i + 1) * TOK_PER_CORE]
        in_maps.append(
            {"xt": np.ascontiguousarray(shard.T), "ctp": ctp}
        )
    return in_maps


def _axon_available():
    """True if this process's jax can see the 8 NeuronCores."""
    try:
        import jax

        return len(jax.devices()) >= N_CORES and jax.default_backend() != "cpu"
    except Exception:
        return False


def _device_affinities_T(x_flat, centroids):
    """Run the 8-core bass kernel; return aff.T [E, N_TOK] float32."""
    if not _axon_available():
        return _device_affinities_T_subprocess(x_flat, centroids)
    from concourse.bass_utils import run_bass_kernel_spmd

    in_maps = _make_in_maps(x_flat, centroids)
    nc = _get_nc()
    res = run_bass_kernel_spmd(nc, in_maps, list(range(N_CORES)))
    shards = []
    for i in range(N_CORES):
        pad = res.results[i]["afft_pad"]  # [128, 512], row 32b+e
        blk = pad.reshape(N_BLK, 32, TOK_BLK)[:, :E, :]
        shards.append(
            blk.transpose(1, 0, 2).reshape(E, TOK_PER_CORE)
        )
    return np.concatenate(shards, axis=1)  # [E, N_TOK]


def _device_affinities_T_subprocess(x_flat, centroids):
    """Fallback when the calling process pinned jax to CPU: run the device
    kernel in a child process where the neuron/axon PJRT plugin can boot."""
    import os
    import subprocess
    import sys
    import tempfile

    here = os.path.dirname(os.path.abspath(__file__))
    with tempfile.TemporaryDirectory() as td:
        np.save(os.path.join(td, "x.npy"), x_flat)
        np.save(os.path.join(td, "c.npy"), centroids)
        prog = (
            "import sys, numpy as np\n"
            f"sys.path.insert(0, {here!r})\n"
            "import kernel as _k\n"
            f"x = np.load({os.path.join(td, 'x.npy')!r})\n"
            f"c = np.load({os.path.join(td, 'c.npy')!r})\n"
            "a = _k._device_affinities_T(x, c)\n"
            f"np.save({os.path.join(td, 'a.npy')!r}, a)\n"
        )
        env = dict(os.environ)
        env.pop("JAX_PLATFORMS", None)
        env["JAX_PLATFORMS"] = "axon"
        subprocess.run(
            [sys.executable, "-c", prog], env=env, check=True,
            stdout=subprocess.DEVNULL, stderr=subprocess.DEVNULL,
        )
        return np.load(os.path.join(td, "a.npy"))


def _balanced_assignment_host(s):
    """Exact host replica of the reference auction on s = scores.T [E, N]."""
    ok = np.isfinite(s)
    if not ok.all():
        fmin = np.min(np.where(ok, s, np.inf))
        s = np.where(ok, s, fmin).astype(np.float32)
    eps = np.maximum(
        np.float32((np.float32(s.max()) - np.float32(s.min())) / np.float32(50.0)),
        np.float32(1e-4),
    )
    E_, N = s.shape
    jpw = N // E_
    rows = np.arange(E_)[:, None]
    jobs_idx = np.arange(N)
    MAX_GREEDY = 100
    HARD_CAP = 200

    value = s.copy()
    cost = np.zeros(N, np.float32)
    prev_bidders = np.zeros(N, np.int32)
    prev_have = np.zeros(N, bool)
    it = 0
    top_index = None
    while it < HARD_CAP:
        order = np.argsort(-value, axis=1, kind="stable")
        top_index = order[:, : jpw + 1]
        top_values = np.take_along_axis(value, top_index, axis=1)
        bid_incr = top_values[:, :jpw] - top_values[:, jpw:] + eps
        bids = np.zeros_like(s)
        bids[rows, top_index[:, :jpw]] = bid_incr
        bids[prev_bidders, jobs_idx] = np.where(
            prev_have, eps, bids[prev_bidders, jobs_idx]
        )
        high_bids = bids.max(axis=0)
        high_bidders = bids.argmax(axis=0).astype(np.int32)
        have_bids = high_bids > 0
        done = bool(np.all(have_bids))
        cost = (cost + high_bids).astype(np.float32)
        value = (s - cost).astype(np.float32)
        if it < MAX_GREEDY:
            upd = np.full(N, np.inf, np.float32)
        else:
            upd = s[high_bidders, jobs_idx]
        value[high_bidders, jobs_idx] = np.where(
            have_bids, upd, value[high_bidders, jobs_idx]
        )
        prev_bidders = high_bidders
        prev_have = have_bids
        it += 1
        if done:
            break
    return top_index[:, :jpw].astype(np.int32)


def kernel(input_features, expert_centroids):
    x_flat = np.ascontiguousarray(
        input_features.reshape(-1, input_features.shape[-1])
    ).astype(np.float32, copy=False)
    afft = _device_affinities_T(x_flat, expert_centroids)  # [E, N]
    top_idx = _balanced_assignment_host(afft)
    top_value = np.take_along_axis(afft, top_idx, axis=1).astype(np.float32)
    return top_idx, top_value


# revision 42
# speedup vs baseline: 1.0530x; 1.0530x over previous
"""BaseLayer MoE gate (balanced assignment) for Trainium2, 8 NeuronCores.

Strategy:
  - The roofline-dominant work is the token->expert affinity matmul
    X[16384, 2048] @ C.T[2048, 16] (reads 134 MB; the kernel is
    HBM-bandwidth bound).  Tokens are sharded 8 ways; each core computes
    aff.T[16, 2048] for its 2048-token shard.
  - Per core: X-shard is fed pre-transposed ([d_model, tok], so the
    d_model contraction lands on SBUF partitions) and streamed as
    sixteen 1MB DMA loads on ONE HWDGE queue (strict FIFO: chunk k
    completes every ~2.5us, so the PE wave for chunk k runs while
    chunk k+1 streams; after the last byte only the last wave + a
    short evacuation remain on the critical path).  The fp32 matmul
    uses PE *column tiling* (tile_position=(0, 32b)) to run the four
    512-token blocks concurrently in the four 32-column PE quadrants
    (fp32 moving costs 4 cycles/row, so without packing the PE would
    be the bottleneck).  Contraction accumulates over 16 k-chunks into
    one PSUM bank.
  - Evacuation: ONE [128, 512] PSUM->SBUF vector copy (DVE time is
    free-dim bound, so copying the whole bank costs the same as one
    quadrant) followed by ONE 256KB store on the drained input queue;
    the host discards the padding rows between quadrants for free.
    A tiny warm-up DMA absorbs the ~1.1us cold-start ramp of the first
    packet on every SDMA engine, and the DCE-keepalive sink store is
    512B so it does not displace input packets on engine 0 (whose
    mid-stream runtime table-refill hiccup already makes it the
    straggler that gates the final chunk).
  - fp32 precision end-to-end is required: the auction's final
    assignment is stable under affinity perturbations up to ~1e-6 but
    flips thousands of indices by 1e-5, which rules out bf16/fp32r
    tricks (verified empirically).
  - Warm-keeping dummy matmuls (one per mid-stream chunk, scratch PSUM
    bank): the PE clock governor (HAM) throttles during long DMA
    waits; the dummies keep the PE boosted so the final (critical-path)
    wave runs at the fast clock.
  - The auction-based balanced assignment operates on the tiny
    [16, 16384] affinity matrix and is an inherently sequential,
    data-dependent while loop (converges in ~11 iterations here); it
    runs on host as an exact bit-level replica of the reference
    semantics (verified to reproduce jax.lax.top_k tie-breaking and the
    full reference trajectory).
"""

import numpy as np

D = 2048
E = 16
N_CORES = 8
TOK_PER_CORE = 2048
N_TOK = N_CORES * TOK_PER_CORE
TOK_BLK = 512
N_BLK = TOK_PER_CORE // TOK_BLK  # 4
K_CHUNKS = D // 128  # 16

_cache = {}


def _build_nc():
    import concourse.tile as tile
    from concourse import bacc, mybir

    f32 = mybir.dt.float32

    nc = bacc.Bacc(
        "TRN2", target_bir_lowering=False, debug=False, num_devices=N_CORES
    )
    xt = nc.declare_dram_parameter("xt", [D, TOK_PER_CORE], f32, isOutput=False)
    # ctp: centroids pre-arranged on host as [128, K_CHUNKS, E]
    ctp = nc.declare_dram_parameter("ctp", [128, K_CHUNKS, E], f32, isOutput=False)
    # raw PSUM-layout output: row 32*b + e holds aff[e, 512b:512b+512]
    # (rows 16..31 of each 32-row quadrant are don't-care padding)
    afft_pad = nc.declare_dram_parameter(
        "afft_pad", [128, TOK_BLK], f32, isOutput=True
    )
    # internal sink that keeps the warm-up dummy matmuls live past DCE;
    # 512B so its store is one descriptor on one SDMA engine (a 32KB sink
    # displaced ~1.5us of engine 0's input stream, measurably gating the
    # final chunk's completion)
    sink = nc.dram_tensor("sink", [E, 8], f32)

    with tile.TileContext(nc) as tc:
        with tc.tile_pool(name="cpool", bufs=1) as cpool, \
             tc.tile_pool(name="xpool", bufs=K_CHUNKS) as xpool, \
             tc.tile_pool(name="opool", bufs=1) as opool, \
             tc.tile_pool(name="spool", bufs=1) as spool, \
             tc.tile_pool(name="psum", bufs=1, space="PSUM") as psum_pool, \
             tc.tile_pool(name="psum2", bufs=2, space="PSUM") as psum2_pool:
            ct_sb = cpool.tile([128, K_CHUNKS, E], f32)
            nc.scalar.dma_start(out=ct_sb[:], in_=ctp[:])
            # One PSUM bank [128, TOK_BLK]; col tile b owns partitions
            # 32b..32b+E (M=16 rows of its 32-partition quadrant).
            ps = psum_pool.tile([128, TOK_BLK], f32)

            # Tiny warm-up transfer: the first packet of a cold HWDGE queue
            # runs ~1.1us instead of ~0.3us on every engine; absorb that
            # ramp on an 8KB throwaway so the real stream starts at line
            # rate.
            warm = spool.tile([128, 16], f32, tag="warm", name="warm")
            nc.sync.dma_start(out=warm[:], in_=xt[0:128, 0:16])

            dummies = []
            for k in range(K_CHUNKS):
                xk = xpool.tile([128, TOK_PER_CORE], f32, tag="xk", name=f"xk_{k}")
                nc.sync.dma_start(out=xk[:], in_=xt[k * 128:(k + 1) * 128, :])
                for b in range(N_BLK):
                    nc.tensor.matmul(
                        ps[32 * b:32 * b + E, :],
                        ct_sb[:, k, :],
                        xk[:, b * TOK_BLK:(b + 1) * TOK_BLK],
                        start=(k == 0), stop=(k == K_CHUNKS - 1),
                        tile_position=(0, 32 * b),
                    )
                if 1 <= k <= K_CHUNKS - 2:
                    # Light warm-keeping dummy matmul (see module
                    # docstring): 128 moving rows is enough to keep HAM
                    # boosted but leaves the PE ~1.8us of slack per chunk,
                    # so the PE never builds a backlog.
                    ps2 = psum2_pool.tile(
                        [128, TOK_BLK], f32, tag="ps2", name=f"ps2_{k}"
                    )
                    nc.tensor.matmul(
                        ps2[0:E, 0:128],
                        ct_sb[:, k, :],
                        xk[:, 0:128],
                        start=True, stop=True,
                        tile_position=(0, 0),
                    )
                    dummies.append(ps2)
                if k == 1:
                    # anchor the DCE-keepalive to the FIRST dummy bank so
                    # this chain retires mid-stream instead of extending
                    # the kernel tail.
                    sb = spool.tile([E, 8], f32, tag="sb", name="sb")
                    nc.vector.tensor_copy(sb[:], dummies[0][0:E, 0:8])
                    nc.scalar.dma_start(out=sink[:], in_=sb[:])

            # Evacuate the whole PSUM bank in ONE [128, 512] vector copy
            # (DVE time is free-dim bound, so this costs the same as one
            # 16-partition copy), then a single 256KB store on the
            # (now-drained) sync queue; the host discards the padding.
            ob = opool.tile([128, TOK_BLK], f32, tag="ob", name="ob")
            nc.vector.tensor_copy(ob[:], ps[:])
            nc.sync.dma_start(out=afft_pad[:], in_=ob[:])
    nc.compile()
    return nc


def _get_nc():
    if "nc" not in _cache:
        _cache["nc"] = _build_nc()
    return _cache["nc"]


def _make_in_maps(x_flat, centroids):
    # [E, D] -> C.T [D, E] -> [K_CHUNKS, 128, E] -> [128, K_CHUNKS, E]
    ctp = np.ascontiguousarray(
        centroids.T.astype(np.float32, copy=False)
        .reshape(K_CHUNKS, 128, E)
        .transpose(1, 0, 2)
    )
    in_maps = []
    for i in range(N_CORES):
        shard = x_flat[i * TOK_PER_CORE:(i + 1) * TOK_PER_CORE]
        in_maps.append(
            {"xt": np.ascontiguousarray(shard.T), "ctp": ctp}
        )
    return in_maps


def _axon_available():
    """True if this process's jax can see the 8 NeuronCores."""
    try:
        import jax

        return len(jax.devices()) >= N_CORES and jax.default_backend() != "cpu"
    except Exception:
        return False


def _device_affinities_T(x_flat, centroids):
    """Run the 8-core bass kernel; return aff.T [E, N_TOK] float32."""
    if not _axon_available():
        return _device_affinities_T_subprocess(x_flat, centroids)
    from concourse.bass_utils import run_bass_kernel_spmd

    in_maps = _make_in_maps(x_flat, centroids)
    nc = _get_nc()
    res = run_bass_kernel_spmd(nc, in_maps, list(range(N_CORES)))
    shards = []
    for i in range(N_CORES):
        pad = res.results[i]["afft_pad"]  # [128, 512], row 32b+e
        blk = pad.reshape(N_BLK, 32, TOK_BLK)[:, :E, :]
        shards.append(
            blk.transpose(1, 0, 2).reshape(E, TOK_PER_CORE)
        )
    return np.concatenate(shards, axis=1)  # [E, N_TOK]


def _device_affinities_T_subprocess(x_flat, centroids):
    """Fallback when the calling process pinned jax to CPU: run the device
    kernel in a child process where the neuron/axon PJRT plugin can boot."""
    import os
    import subprocess
    import sys
    import tempfile

    here = os.path.dirname(os.path.abspath(__file__))
    with tempfile.TemporaryDirectory() as td:
        np.save(os.path.join(td, "x.npy"), x_flat)
        np.save(os.path.join(td, "c.npy"), centroids)
        prog = (
            "import sys, numpy as np\n"
            f"sys.path.insert(0, {here!r})\n"
            "import kernel as _k\n"
            f"x = np.load({os.path.join(td, 'x.npy')!r})\n"
            f"c = np.load({os.path.join(td, 'c.npy')!r})\n"
            "a = _k._device_affinities_T(x, c)\n"
            f"np.save({os.path.join(td, 'a.npy')!r}, a)\n"
        )
        env = dict(os.environ)
        env.pop("JAX_PLATFORMS", None)
        env["JAX_PLATFORMS"] = "axon"
        subprocess.run(
            [sys.executable, "-c", prog], env=env, check=True,
            stdout=subprocess.DEVNULL, stderr=subprocess.DEVNULL,
        )
        return np.load(os.path.join(td, "a.npy"))


def _balanced_assignment_host(s):
    """Exact host replica of the reference auction on s = scores.T [E, N]."""
    ok = np.isfinite(s)
    if not ok.all():
        fmin = np.min(np.where(ok, s, np.inf))
        s = np.where(ok, s, fmin).astype(np.float32)
    eps = np.maximum(
        np.float32((np.float32(s.max()) - np.float32(s.min())) / np.float32(50.0)),
        np.float32(1e-4),
    )
    E_, N = s.shape
    jpw = N // E_
    rows = np.arange(E_)[:, None]
    jobs_idx = np.arange(N)
    MAX_GREEDY = 100
    HARD_CAP = 200

    value = s.copy()
    cost = np.zeros(N, np.float32)
    prev_bidders = np.zeros(N, np.int32)
    prev_have = np.zeros(N, bool)
    it = 0
    top_index = None
    while it < HARD_CAP:
        order = np.argsort(-value, axis=1, kind="stable")
        top_index = order[:, : jpw + 1]
        top_values = np.take_along_axis(value, top_index, axis=1)
        bid_incr = top_values[:, :jpw] - top_values[:, jpw:] + eps
        bids = np.zeros_like(s)
        bids[rows, top_index[:, :jpw]] = bid_incr
        bids[prev_bidders, jobs_idx] = np.where(
            prev_have, eps, bids[prev_bidders, jobs_idx]
        )
        high_bids = bids.max(axis=0)
        high_bidders = bids.argmax(axis=0).astype(np.int32)
        have_bids = high_bids > 0
        done = bool(np.all(have_bids))
        cost = (cost + high_bids).astype(np.float32)
        value = (s - cost).astype(np.float32)
        if it < MAX_GREEDY:
            upd = np.full(N, np.inf, np.float32)
        else:
            upd = s[high_bidders, jobs_idx]
        value[high_bidders, jobs_idx] = np.where(
            have_bids, upd, value[high_bidders, jobs_idx]
        )
        prev_bidders = high_bidders
        prev_have = have_bids
        it += 1
        if done:
            break
    return top_index[:, :jpw].astype(np.int32)


def kernel(input_features, expert_centroids):
    x_flat = np.ascontiguousarray(
        input_features.reshape(-1, input_features.shape[-1])
    ).astype(np.float32, copy=False)
    afft = _device_affinities_T(x_flat, expert_centroids)  # [E, N]
    top_idx = _balanced_assignment_host(afft)
    top_value = np.take_along_axis(afft, top_idx, axis=1).astype(np.float32)
    return top_idx, top_value
